# revision 11
# baseline (speedup 1.0000x reference)
"""Trainium2 Bass kernel for top-2 MoE MLP (SwiGLU experts).

Problem shapes (hardcoded):
  hidden_states [2, 1024, 1024] f32, gate_w [1024, 8] f32,
  w_gate/w_up [8, 1024, 2816] f32, w_down [8, 2816, 1024] f32, top_k = 2.

Strategy: expert-parallel over 8 NeuronCores (1 expert per core).
  - Router (x @ gate_w, softmax, top-2, renorm) computed on host with jax
    on CPU, exactly mirroring the reference implementation.
  - Tokens are dispatched (gathered) to their experts on the host; each
    core computes the SwiGLU MLP for the tokens routed to its expert in
    fp16 (fp32 PSUM accumulation), with all expert weights SBUF-resident.
  - Per-token routing weights are applied on the host during the
    scatter-add combine (fp32).

Schedule notes (device side), from HW measurements (see memory notes):
  - Matmul sustained cost ~= N/2.4GHz streaming + ~53 ns serialized
    LDWEIGHTS per instruction (the load does not pipeline under the
    previous matmul).  The fp16 floor for this tiling is ~177 us/rep.
  - Token capacity C is split into two near-equal 16B-aligned chunks
    (552 -> 288+264) instead of 512+40: both LDWEIGHTS buy a long
    stream, and the split point keeps SBUF reads 16B-aligned.
  - 4 PSUM accumulation groups always in flight (4 tags x bufs=2 =
    8 banks): same-bank revisit at distance 2 stalls PE ~40 ns/MM, so
    phase B processes output tiles in pairs.
  - All DRAM tensors are host pre-tiled to [128, X] layouts that map to
    1 contiguous descriptor per partition per DMA; y-output DMAs ride
    the ACT HWDGE ring so they never queue ahead of the next rep's xt
    load on the SP ring (rings are FIFO per issuing engine).
"""

import numpy as np

import concourse.bacc as bacc
import concourse.tile as tile
import concourse.mybir as mybir
from concourse.bass_utils import run_bass_kernel_spmd

B, S, H = 2, 1024, 1024
E, F, TOPK = 8, 2816, 2
T = B * S
P = 128
NK = H // P    # 8 contraction tiles over H
NF = F // P    # 22 tiles over F
NH = H // P    # 8 output tiles over H
F16 = mybir.dt.float16
F32 = mybir.dt.float32

LAST_RESULTS = None  # BassKernelResults of the most recent run (for test harness)

_NC_CACHE = {}


def _to_f16(a: np.ndarray) -> np.ndarray:
    return np.asarray(a, dtype=np.float32).astype(np.float16)


def _routing(x: np.ndarray, gate_w: np.ndarray):
    """Replicates the reference router on CPU jax: softmax fp32, top-2,
    renormalized weights. Returns (sel [T,2] int, top_w [T,2] f32)."""
    import jax
    import jax.numpy as jnp

    cpu = jax.local_devices(backend="cpu")[0]
    with jax.default_device(cpu):
        logits = jnp.asarray(x) @ jnp.asarray(gate_w)
        probs = jax.nn.softmax(logits.astype(jnp.float32), axis=-1)
        top_w, sel = jax.lax.top_k(probs, TOPK)
        top_w = top_w / top_w.sum(axis=-1, keepdims=True)
    return np.asarray(sel), np.asarray(top_w, dtype=np.float32)


def _chunks(C: int):
    if C <= 512:
        return [(0, C)]
    h = (C // 2 + 7) & ~7  # split point 8-elem aligned -> 16B fp16 offsets
    return [(0, h), (h, C - h)]


def _build_nc(C: int, reps: int = 1):
    """Per-core Bass program for capacity-C token batches.

    DRAM inputs (per core, all fp16, host pre-tiled):
      xt [128, NK*C]      xt[p, ko*C + c]            = x[tok c, ko*128+p]
      wg [128, NF*NK*128] wg[p, ((fo*NK)+ko)*128+fi] = w_gate[ko*128+p, fo*128+fi]
      wu [128, NF*NK*128] (same layout as wg)
      wd [128, NH*NF*128] wd[p, ((ho*NF)+fo)*128+hi] = w_down[fo*128+p, ho*128+hi]
    Output:
      y  [128, NH*C]      y[p, ho*C + c]             = out[ho*128+p, tok c]
    """
    nc = bacc.Bacc("TRN2", target_bir_lowering=False, debug=False, num_devices=8)

    xt_d = nc.dram_tensor("xt", [P, NK * C], F16, kind="ExternalInput").ap()
    wg_d = nc.dram_tensor("wg", [P, NF * NK * P], F16, kind="ExternalInput").ap()
    wu_d = nc.dram_tensor("wu", [P, NF * NK * P], F16, kind="ExternalInput").ap()
    wd_d = nc.dram_tensor("wd", [P, NH * NF * P], F16, kind="ExternalInput").ap()
    y_d = nc.dram_tensor("y", [P, NH * C], F16, kind="ExternalOutput").ap()

    CH = _chunks(C)
    CW = max(w for _, w in CH)  # psum tile width (one bank if <= 512)

    with tile.TileContext(nc) as tc:
        with (
            tc.tile_pool(name="weights", bufs=1) as wpool,
            tc.tile_pool(name="acts", bufs=1) as apool,
            tc.tile_pool(name="tmps", bufs=4) as tpool,
            tc.tile_pool(name="outs", bufs=4) as opool,
            tc.tile_pool(name="psum", bufs=2, space="PSUM") as pspool,
        ):
            def body():
                xt_sb = apool.tile([P, NK * C], F16, tag="xt")
                wg_sb = wpool.tile([P, NF * NK * P], F16, tag="wg")
                wu_sb = wpool.tile([P, NF * NK * P], F16, tag="wu")
                wd_sb = wpool.tile([P, NH * NF * P], F16, tag="wd")
                ht_sb = apool.tile([P, NF * C], F16, tag="ht")

                # One contiguous descriptor per partition per dma_start.
                # xt + wg/wu ride the SP HWDGE ring; wd + y outputs ride the
                # ACT ring so end-of-rep y writes never block the next rep's
                # xt load (rings are FIFO per issuing engine).
                nc.sync.dma_start(xt_sb[:], xt_d[:])
                for a, b in ((0, 2), (2, 6), (6, 10), (10, 14), (14, 18),
                             (18, NF)):
                    s = slice(a * NK * P, b * NK * P)
                    nc.sync.dma_start(wg_sb[:, s], wg_d[:, s])
                    nc.sync.dma_start(wu_sb[:, s], wu_d[:, s])
                for a, b in ((0, 2), (2, 4), (4, 6), (6, NH)):
                    s = slice(a * NF * P, b * NF * P)
                    nc.sync.dma_start(wd_sb[:, s], wd_d[:, s])

                # Phase A: g = x @ wg, u = x @ wu, ht = silu(g) * u
                # ht layout [f, tok] so phase B contracts f on partitions.
                for fo in range(NF):
                    pg = [pspool.tile([P, CW], F32, tag=f"pg{i}",
                                      name=f"pg{i}") for i in range(len(CH))]
                    pu = [pspool.tile([P, CW], F32, tag=f"pu{i}",
                                      name=f"pu{i}") for i in range(len(CH))]
                    for ko in range(NK):
                        w0 = ((fo * NK) + ko) * P
                        lg = wg_sb[:, w0:w0 + P]
                        lu = wu_sb[:, w0:w0 + P]
                        for i, (c0, cw) in enumerate(CH):
                            nc.tensor.matmul(
                                pg[i][:, 0:cw], lg,
                                xt_sb[:, ko * C + c0: ko * C + c0 + cw],
                                start=(ko == 0), stop=(ko == NK - 1),
                            )
                        for i, (c0, cw) in enumerate(CH):
                            nc.tensor.matmul(
                                pu[i][:, 0:cw], lu,
                                xt_sb[:, ko * C + c0: ko * C + c0 + cw],
                                start=(ko == 0), stop=(ko == NK - 1),
                            )
                    for i, (c0, cw) in enumerate(CH):
                        tmp = tpool.tile([P, CW], F32, name=f"tmp{i}")
                        nc.scalar.activation(
                            tmp[:, 0:cw], pg[i][:, 0:cw],
                            mybir.ActivationFunctionType.Silu,
                        )
                        nc.vector.tensor_mul(
                            ht_sb[:, fo * C + c0: fo * C + c0 + cw],
                            tmp[:, 0:cw], pu[i][:, 0:cw],
                        )

                # Phase B: yT = wd.T @ ht  (h on partitions, tokens moving).
                # ho processed in pairs -> 4 PSUM banks round-robin, so
                # same-bank revisit distance stays 4 (distance 2 stalls PE
                # ~150 ns/MM on the PSUM accumulate path).
                for hp in range(0, NH, 2):
                    pys = []
                    for j, tagset in ((0, ("pg0", "pg1")), (1, ("pu0", "pu1"))):
                        pys.append([pspool.tile([P, CW], F32, tag=tagset[i],
                                                name=f"py{j}{i}")
                                    for i in range(len(CH))])
                    for fo in range(NF):
                        for j in range(2):
                            w0 = (((hp + j) * NF) + fo) * P
                            lw = wd_sb[:, w0:w0 + P]
                            for i, (c0, cw) in enumerate(CH):
                                nc.tensor.matmul(
                                    pys[j][i][:, 0:cw], lw,
                                    ht_sb[:, fo * C + c0: fo * C + c0 + cw],
                                    start=(fo == 0), stop=(fo == NF - 1),
                                )
                    for j in range(2):
                        ot = opool.tile([P, C], F16, name=f"ot{j}")
                        for i, (c0, cw) in enumerate(CH):
                            nc.vector.tensor_copy(ot[:, c0:c0 + cw],
                                                  pys[j][i][:, 0:cw])
                        ho = hp + j
                        nc.scalar.dma_start(y_d[:, ho * C:(ho + 1) * C],
                                            ot[:])

            if reps == 1:
                body()
            else:
                with tc.For_i(0, reps, 1):
                    body()

    nc.compile()
    return nc


def _tile_xt(xe_T: np.ndarray, C: int) -> np.ndarray:
    """[H, m] fp16 token features -> [128, NK*C] padded pre-tiled."""
    m = xe_T.shape[1]
    out = np.zeros((P, NK, C), dtype=np.float16)
    out[:, :, :m] = xe_T.reshape(NK, P, m).transpose(1, 0, 2)
    return out.reshape(P, NK * C)


def _tile_w_in(w: np.ndarray) -> np.ndarray:
    """[H, F] -> [128, NF*NK*128]: w_t[p, ((fo*NK)+ko)*128+fi] = w[ko*128+p, fo*128+fi]"""
    return np.ascontiguousarray(
        w.reshape(NK, P, NF, P).transpose(1, 2, 0, 3)
    ).reshape(P, NF * NK * P)


def _tile_w_out(w: np.ndarray) -> np.ndarray:
    """[F, H] -> [128, NH*NF*128]: w_t[p, ((ho*NF)+fo)*128+hi] = w[fo*128+p, ho*128+hi]"""
    return np.ascontiguousarray(
        w.reshape(NF, P, NH, P).transpose(1, 2, 0, 3)
    ).reshape(P, NH * NF * P)


def kernel(hidden_states, gate_w, w_gate, w_up, w_down):
    global LAST_RESULTS

    x = np.ascontiguousarray(np.asarray(hidden_states), dtype=np.float32).reshape(T, H)
    gate_w = np.asarray(gate_w, dtype=np.float32)

    sel, top_w = _routing(x, gate_w)

    # Group (token, slot) pairs by expert.
    flat_sel = sel.ravel()                       # [T*2]
    flat_tok = np.repeat(np.arange(T), TOPK)     # [T*2]
    flat_w = top_w.ravel()                       # [T*2]
    order = np.argsort(flat_sel, kind="stable")
    counts = np.bincount(flat_sel, minlength=E)
    starts = np.concatenate([[0], np.cumsum(counts)])
    toks = [flat_tok[order[starts[e]:starts[e + 1]]] for e in range(E)]
    wts = [flat_w[order[starts[e]:starts[e + 1]]] for e in range(E)]

    C = max(128, int(-(-counts.max() // 8)) * 8)  # capacity, multiple of 8

    xt_all = np.zeros((E, P, NK * C), dtype=np.float16)
    for e in range(E):
        if counts[e]:
            xt_all[e] = _tile_xt(_to_f16(x[toks[e]].T), C)

    wg_t = np.stack([_tile_w_in(_to_f16(w_gate[e])) for e in range(E)])
    wu_t = np.stack([_tile_w_in(_to_f16(w_up[e])) for e in range(E)])
    wd_t = np.stack([_tile_w_out(_to_f16(w_down[e])) for e in range(E)])

    if C not in _NC_CACHE:
        _NC_CACHE[C] = _build_nc(C, 1)
    nc = _NC_CACHE[C]

    in_maps = [
        {"xt": xt_all[e], "wg": wg_t[e], "wu": wu_t[e], "wd": wd_t[e]}
        for e in range(E)
    ]
    res = run_bass_kernel_spmd(nc, in_maps, core_ids=list(range(E)))
    LAST_RESULTS = res
    globals()["LAST_IN_MAPS"], globals()["LAST_C"] = in_maps, C

    out = np.zeros((T, H), dtype=np.float32)
    for e in range(E):
        m = counts[e]
        if m:
            y_t = np.asarray(res.results[e]["y"], dtype=np.float32)
            y_e = y_t.reshape(P, NH, C).transpose(1, 0, 2).reshape(H, C)[:, :m].T
            out[toks[e]] += wts[e][:, None] * y_e

    return out.reshape(B, S, H)


# revision 13
# speedup vs baseline: 1.0690x; 1.0690x over previous
"""Trainium2 Bass kernel for top-2 MoE MLP (SwiGLU experts).

Problem shapes (hardcoded):
  hidden_states [2, 1024, 1024] f32, gate_w [1024, 8] f32,
  w_gate/w_up [8, 1024, 2816] f32, w_down [8, 2816, 1024] f32, top_k = 2.

Strategy: expert-parallel over 8 NeuronCores (1 expert per core).
  - Router (x @ gate_w, softmax, top-2, renorm) computed on host with jax
    on CPU, exactly mirroring the reference implementation.
  - Tokens are dispatched (gathered) to their experts on the host; each
    core computes the SwiGLU MLP for the tokens routed to its expert in
    fp16 (fp32 PSUM accumulation), with all expert weights SBUF-resident.
  - Per-token routing weights are applied on the host during the
    scatter-add combine (fp32).

Schedule notes (device side), from HW measurements (see memory notes):
  - Matmul sustained cost ~= N/2.4GHz streaming + ~53 ns serialized
    LDWEIGHTS per instruction (the load does not pipeline under the
    previous matmul).  The fp16 floor for this tiling is ~177 us/rep.
  - Token capacity C is split into two near-equal 16B-aligned chunks
    (552 -> 280+272) instead of 512+40: both LDWEIGHTS buy a long
    stream, and the split point keeps SBUF reads 16B-aligned.
  - 4 PSUM accumulation groups always in flight (4 tags x bufs=2 =
    8 banks): same-bank revisit at distance 2 stalls PE ~40 ns/MM, so
    phase B processes output tiles in pairs.
  - All DRAM tensors are host pre-tiled to [128, X] layouts that map to
    1 contiguous descriptor per partition per DMA; y-output DMAs ride
    the ACT HWDGE ring so they never queue ahead of the next rep's xt
    load on the SP ring (rings are FIFO per issuing engine).
"""

import numpy as np

import concourse.bacc as bacc
import concourse.tile as tile
import concourse.mybir as mybir
from concourse.bass_utils import run_bass_kernel_spmd

B, S, H = 2, 1024, 1024
E, F, TOPK = 8, 2816, 2
T = B * S
P = 128
NK = H // P    # 8 contraction tiles over H
NF = F // P    # 22 tiles over F
NH = H // P    # 8 output tiles over H
F16 = mybir.dt.float16
F32 = mybir.dt.float32

LAST_RESULTS = None  # BassKernelResults of the most recent run (for test harness)

_NC_CACHE = {}


def _to_f16(a: np.ndarray) -> np.ndarray:
    return np.asarray(a, dtype=np.float32).astype(np.float16)


def _routing(x: np.ndarray, gate_w: np.ndarray):
    """Replicates the reference router on CPU jax: softmax fp32, top-2,
    renormalized weights. Returns (sel [T,2] int, top_w [T,2] f32)."""
    import jax
    import jax.numpy as jnp

    cpu = jax.local_devices(backend="cpu")[0]
    with jax.default_device(cpu):
        logits = jnp.asarray(x) @ jnp.asarray(gate_w)
        probs = jax.nn.softmax(logits.astype(jnp.float32), axis=-1)
        top_w, sel = jax.lax.top_k(probs, TOPK)
        top_w = top_w / top_w.sum(axis=-1, keepdims=True)
    return np.asarray(sel), np.asarray(top_w, dtype=np.float32)


def _chunks(C: int):
    if C <= 512:
        return [(0, C)]
    h = (C // 2 + 7) & ~7  # split point 8-elem aligned -> 16B fp16 offsets
    return [(0, h), (h, C - h)]


def _build_nc(C: int, reps: int = 1):
    """Per-core Bass program for capacity-C token batches.

    DRAM inputs (per core, all fp16, host pre-tiled):
      xt [128, NK*C]      xt[p, ko*C + c]            = x[tok c, ko*128+p]
      wg [128, NF*NK*128] wg[p, ((fo*NK)+ko)*128+fi] = w_gate[ko*128+p, fo*128+fi]
      wu [128, NF*NK*128] (same layout as wg)
      wd [128, NH*NF*128] wd[p, ((ho*NF)+fo)*128+hi] = w_down[fo*128+p, ho*128+hi]
    Output:
      y  [128, NH*C]      y[p, ho*C + c]             = out[ho*128+p, tok c]
    """
    nc = bacc.Bacc("TRN2", target_bir_lowering=False, debug=False, num_devices=8)

    xt_d = nc.dram_tensor("xt", [P, NK * C], F16, kind="ExternalInput").ap()
    wg_d = nc.dram_tensor("wg", [P, NF * NK * P], F16, kind="ExternalInput").ap()
    wu_d = nc.dram_tensor("wu", [P, NF * NK * P], F16, kind="ExternalInput").ap()
    wd_d = nc.dram_tensor("wd", [P, NH * NF * P], F16, kind="ExternalInput").ap()
    y_d = nc.dram_tensor("y", [P, NH * C], F16, kind="ExternalOutput").ap()

    CH = _chunks(C)
    CW = max(w for _, w in CH)  # psum tile width (one bank if <= 512)

    with tile.TileContext(nc) as tc:
        with (
            tc.tile_pool(name="weights", bufs=1) as wpool,
            tc.tile_pool(name="acts", bufs=1) as apool,
            tc.tile_pool(name="tmps", bufs=4) as tpool,
            tc.tile_pool(name="outs", bufs=4) as opool,
            tc.tile_pool(name="psum", bufs=2, space="PSUM") as pspool,
        ):
            def body():
                xt_sb = apool.tile([P, NK * C], F16, tag="xt")
                wg_sb = wpool.tile([P, NF * NK * P], F16, tag="wg")
                wu_sb = wpool.tile([P, NF * NK * P], F16, tag="wu")
                wd_sb = wpool.tile([P, NH * NF * P], F16, tag="wd")
                ht_sb = apool.tile([P, NF * C], F16, tag="ht")

                # One contiguous descriptor per partition per dma_start.
                # xt + wg/wu ride the SP HWDGE ring; wd + y outputs ride the
                # ACT ring so end-of-rep y writes never block the next rep's
                # xt load (rings are FIFO per issuing engine).
                nc.sync.dma_start(xt_sb[:], xt_d[:])
                for a, b in ((0, 2), (2, 6), (6, 10), (10, 14), (14, 18),
                             (18, NF)):
                    s = slice(a * NK * P, b * NK * P)
                    nc.sync.dma_start(wg_sb[:, s], wg_d[:, s])
                    nc.sync.dma_start(wu_sb[:, s], wu_d[:, s])
                for a, b in ((0, 2), (2, 4), (4, 6), (6, NH)):
                    s = slice(a * NF * P, b * NF * P)
                    nc.scalar.dma_start(wd_sb[:, s], wd_d[:, s])

                # Phase A: g = x @ wg, u = x @ wu, ht = silu(g) * u
                # ht layout [f, tok] so phase B contracts f on partitions.
                for fo in range(NF):
                    pg = [pspool.tile([P, CW], F32, tag=f"pg{i}",
                                      name=f"pg{i}") for i in range(len(CH))]
                    pu = [pspool.tile([P, CW], F32, tag=f"pu{i}",
                                      name=f"pu{i}") for i in range(len(CH))]
                    for ko in range(NK):
                        w0 = ((fo * NK) + ko) * P
                        lg = wg_sb[:, w0:w0 + P]
                        lu = wu_sb[:, w0:w0 + P]
                        for i, (c0, cw) in enumerate(CH):
                            nc.tensor.matmul(
                                pg[i][:, 0:cw], lg,
                                xt_sb[:, ko * C + c0: ko * C + c0 + cw],
                                start=(ko == 0), stop=(ko == NK - 1),
                            )
                        for i, (c0, cw) in enumerate(CH):
                            nc.tensor.matmul(
                                pu[i][:, 0:cw], lu,
                                xt_sb[:, ko * C + c0: ko * C + c0 + cw],
                                start=(ko == 0), stop=(ko == NK - 1),
                            )
                    for i, (c0, cw) in enumerate(CH):
                        tmp = tpool.tile([P, CW], F32, name=f"tmp{i}")
                        nc.scalar.activation(
                            tmp[:, 0:cw], pg[i][:, 0:cw],
                            mybir.ActivationFunctionType.Silu,
                        )
                        nc.vector.tensor_mul(
                            ht_sb[:, fo * C + c0: fo * C + c0 + cw],
                            tmp[:, 0:cw], pu[i][:, 0:cw],
                        )

                # Phase B: yT = wd.T @ ht  (h on partitions, tokens moving).
                # ho processed in pairs -> 4 PSUM banks round-robin, so
                # same-bank revisit distance stays 4 (distance 2 stalls PE
                # ~150 ns/MM on the PSUM accumulate path).
                for hp in range(0, NH, 2):
                    pys = []
                    for j, tagset in ((0, ("pg0", "pg1")), (1, ("pu0", "pu1"))):
                        pys.append([pspool.tile([P, CW], F32, tag=tagset[i],
                                                name=f"py{j}{i}")
                                    for i in range(len(CH))])
                    for fo in range(NF):
                        for j in range(2):
                            w0 = (((hp + j) * NF) + fo) * P
                            lw = wd_sb[:, w0:w0 + P]
                            for i, (c0, cw) in enumerate(CH):
                                nc.tensor.matmul(
                                    pys[j][i][:, 0:cw], lw,
                                    ht_sb[:, fo * C + c0: fo * C + c0 + cw],
                                    start=(fo == 0), stop=(fo == NF - 1),
                                )
                    for j in range(2):
                        ot = opool.tile([P, C], F16, name=f"ot{j}")
                        for i, (c0, cw) in enumerate(CH):
                            nc.vector.tensor_copy(ot[:, c0:c0 + cw],
                                                  pys[j][i][:, 0:cw])
                        ho = hp + j
                        nc.scalar.dma_start(y_d[:, ho * C:(ho + 1) * C],
                                            ot[:])

            if reps == 1:
                body()
            else:
                with tc.For_i(0, reps, 1):
                    body()

    nc.compile()
    return nc


def _tile_xt(xe_T: np.ndarray, C: int) -> np.ndarray:
    """[H, m] fp16 token features -> [128, NK*C] padded pre-tiled."""
    m = xe_T.shape[1]
    out = np.zeros((P, NK, C), dtype=np.float16)
    out[:, :, :m] = xe_T.reshape(NK, P, m).transpose(1, 0, 2)
    return out.reshape(P, NK * C)


def _tile_w_in(w: np.ndarray) -> np.ndarray:
    """[H, F] -> [128, NF*NK*128]: w_t[p, ((fo*NK)+ko)*128+fi] = w[ko*128+p, fo*128+fi]"""
    return np.ascontiguousarray(
        w.reshape(NK, P, NF, P).transpose(1, 2, 0, 3)
    ).reshape(P, NF * NK * P)


def _tile_w_out(w: np.ndarray) -> np.ndarray:
    """[F, H] -> [128, NH*NF*128]: w_t[p, ((ho*NF)+fo)*128+hi] = w[fo*128+p, ho*128+hi]"""
    return np.ascontiguousarray(
        w.reshape(NF, P, NH, P).transpose(1, 2, 0, 3)
    ).reshape(P, NH * NF * P)


def kernel(hidden_states, gate_w, w_gate, w_up, w_down):
    global LAST_RESULTS

    x = np.ascontiguousarray(np.asarray(hidden_states), dtype=np.float32).reshape(T, H)
    gate_w = np.asarray(gate_w, dtype=np.float32)

    sel, top_w = _routing(x, gate_w)

    # Group (token, slot) pairs by expert.
    flat_sel = sel.ravel()                       # [T*2]
    flat_tok = np.repeat(np.arange(T), TOPK)     # [T*2]
    flat_w = top_w.ravel()                       # [T*2]
    order = np.argsort(flat_sel, kind="stable")
    counts = np.bincount(flat_sel, minlength=E)
    starts = np.concatenate([[0], np.cumsum(counts)])
    toks = [flat_tok[order[starts[e]:starts[e + 1]]] for e in range(E)]
    wts = [flat_w[order[starts[e]:starts[e + 1]]] for e in range(E)]

    C = max(128, int(-(-counts.max() // 8)) * 8)  # capacity, multiple of 8

    xt_all = np.zeros((E, P, NK * C), dtype=np.float16)
    for e in range(E):
        if counts[e]:
            xt_all[e] = _tile_xt(_to_f16(x[toks[e]].T), C)

    wg_t = np.stack([_tile_w_in(_to_f16(w_gate[e])) for e in range(E)])
    wu_t = np.stack([_tile_w_in(_to_f16(w_up[e])) for e in range(E)])
    wd_t = np.stack([_tile_w_out(_to_f16(w_down[e])) for e in range(E)])

    if C not in _NC_CACHE:
        _NC_CACHE[C] = _build_nc(C, 1)
    nc = _NC_CACHE[C]

    in_maps = [
        {"xt": xt_all[e], "wg": wg_t[e], "wu": wu_t[e], "wd": wd_t[e]}
        for e in range(E)
    ]
    res = run_bass_kernel_spmd(nc, in_maps, core_ids=list(range(E)))
    LAST_RESULTS = res
    globals()["LAST_IN_MAPS"], globals()["LAST_C"] = in_maps, C

    out = np.zeros((T, H), dtype=np.float32)
    for e in range(E):
        m = counts[e]
        if m:
            y_t = np.asarray(res.results[e]["y"], dtype=np.float32)
            y_e = y_t.reshape(P, NH, C).transpose(1, 0, 2).reshape(H, C)[:, :m].T
            out[toks[e]] += wts[e][:, None] * y_e

    return out.reshape(B, S, H)
